# revision 1
# baseline (speedup 1.0000x reference)
"""GNN message-passing kernel for Trainium2 (Bass/Tile), 8-core SPMD.

Sharding: edges sharded by receiver range (edge/data parallel, no collectives).
Core c owns receivers in [c*NPC, (c+1)*NPC). Each core:
  phase A: P1  = nf @ W1            (full table, all cores identical)
  phase B: P2b = nf_loc @ W2 + b    (local shard only)
  phase C: per 128-node window, per 128-edge tile:
           msg = relu(P1[s] + P2b[r] + efT.T @ W3)
           aggr[window] += S_tile.T @ msg          (S = one-hot of ranks)
           out = LayerNorm(aggr + nf_shard)
All matmuls/adds in fp32 (bit-accuracy limited only by summation order).
"""

import numpy as np

import concourse.bacc as bacc
import concourse.tile as tile
import concourse.mybir as mybir
import concourse.bass as bass
from concourse.tile_rust import add_dep_helper

F32 = mybir.dt.float32
I16 = mybir.dt.int16


# ----------------------------------------------------------------------------
# Host-side preparation
# ----------------------------------------------------------------------------

def host_prep(node_features, senders, receivers, edge_features, W, b, ln_w, ln_b,
              n_cores=8, hi_base=32768):
    N, H = node_features.shape
    E = senders.shape[0]
    assert H == 128
    NPC = N // n_cores                      # nodes per core
    WPC = (NPC + 127) // 128                # windows per core
    NPC_PAD = WPC * 128
    NT_GLOBAL = (N + 127) // 128            # node tiles for P1 table
    N_PAD = NT_GLOBAL * 128
    HI_BASE = hi_base                       # int16 gather split point

    node_features = np.asarray(node_features, np.float32)
    senders = np.asarray(senders, np.int32)
    receivers = np.asarray(receivers, np.int32)
    edge_features = np.asarray(edge_features, np.float32)

    core_of_edge = receivers // NPC
    core_of_edge = np.minimum(core_of_edge, n_cores - 1)  # guard (shouldn't hit)

    # ---- pass 1: per-core, per-window lo/hi counts --------------------------
    per_core = []
    lo_cnt = np.zeros((n_cores, WPC), np.int64)
    hi_cnt = np.zeros((n_cores, WPC), np.int64)
    for c in range(n_cores):
        sel = np.nonzero(core_of_edge == c)[0]
        r_loc = receivers[sel] - c * NPC
        s = senders[sel]
        w = r_loc // 128
        hi = (s >= HI_BASE)
        order = np.lexsort((hi, w))
        sel, r_loc, s, w, hi = sel[order], r_loc[order], s[order], w[order], hi[order]
        lo_cnt[c] = np.bincount(w[~hi], minlength=WPC)
        hi_cnt[c] = np.bincount(w[hi], minlength=WPC)
        per_core.append((sel, r_loc, s, w, hi))

    T_lo = ((lo_cnt.max(axis=0) + 127) // 128).astype(np.int64)
    T_hi = ((hi_cnt.max(axis=0) + 127) // 128).astype(np.int64)
    T_w = T_lo + T_hi                       # tiles per window (shared all cores)
    NT = int(T_w.sum())                     # total tiles per core
    E_PAD = NT * 128

    # window tile-base offsets in the padded stream (tile units)
    tile_base = np.concatenate(([0], np.cumsum(T_w)[:-1]))
    lo_tile_base = np.concatenate(([0], np.cumsum(T_lo)[:-1]))
    hi_tile_base = np.concatenate(([0], np.cumsum(T_hi)[:-1]))
    L_LO = int(T_lo.sum()) * 128
    L_HI = max(int(T_hi.sum()) * 128, 128)

    def wrap_idx(arr):
        """int16 stream -> [128, L/16] wrapped layout (replicated per 16 rows)."""
        L = arr.shape[0]
        assert L % 16 == 0
        w16 = arr.reshape(-1, 16).T.astype(np.int16)   # [16, L/16]
        return np.ascontiguousarray(np.tile(w16, (8, 1)))

    structure = dict(N=N, H=H, E=E, NPC=NPC, WPC=WPC, NPC_PAD=NPC_PAD,
                     NT_GLOBAL=NT_GLOBAL, N_PAD=N_PAD, NT=NT, E_PAD=E_PAD,
                     T_lo=T_lo, T_hi=T_hi, T_w=T_w, tile_base=tile_base,
                     lo_tile_base=lo_tile_base, hi_tile_base=hi_tile_base,
                     L_LO=L_LO, L_HI=L_HI, HI_BASE=HI_BASE)

    # ---- shared (core-independent) inputs -----------------------------------
    nfT = np.zeros((128, N_PAD), np.float32)
    nfT[:, :N] = node_features.T
    iota_row = np.broadcast_to(np.arange(128, dtype=np.float32), (128, 128)).copy()
    shared = {
        "nfT": nfT,
        "W1": np.ascontiguousarray(W[0:128], np.float32),
        "W2": np.ascontiguousarray(W[128:256], np.float32),
        "W3": np.ascontiguousarray(W[256:384], np.float32),
        "b_bc": np.broadcast_to(np.asarray(b, np.float32), (128, 128)).copy(),
        "b_col": np.asarray(b, np.float32).reshape(128, 1).copy(),
        "ident": np.eye(128, dtype=np.float32),
        "lnw_bc": np.broadcast_to(np.asarray(ln_w, np.float32), (128, 128)).copy(),
        "lnb_bc": np.broadcast_to(np.asarray(ln_b, np.float32), (128, 128)).copy(),
        "iota": iota_row,
    }

    # ---- pass 2: per-core padded streams ------------------------------------
    in_maps = []
    for c in range(n_cores):
        sel, r_loc, s, w, hi = per_core[c]
        Ec = sel.shape[0]
        # within-window within-group running index
        grp = w * 2 + hi.astype(np.int64)            # sorted by (w, hi)
        starts = np.concatenate(([0], np.nonzero(np.diff(grp))[0] + 1))
        grp_start_per_edge = np.repeat(starts, np.diff(np.concatenate((starts, [Ec]))))
        j = np.arange(Ec) - grp_start_per_edge
        pos = np.where(
            hi,
            (tile_base[w] + T_lo[w]) * 128 + j,
            tile_base[w] * 128 + j,
        )

        ef_pad = np.zeros((E_PAD, 128), np.float32)
        ef_pad[pos] = edge_features[sel]
        efT = np.ascontiguousarray(ef_pad.T)

        rank = np.full(E_PAD, -1.0, np.float32)
        rank[pos] = (r_loc - w * 128).astype(np.float32)
        rankT = np.ascontiguousarray(rank.reshape(NT, 128).T)   # [128, NT]

        idx_lo = np.zeros(L_LO, np.int64)
        lo_pos = lo_tile_base[w[~hi]] * 128 + j[~hi]
        idx_lo[lo_pos] = s[~hi]
        idx_hi = np.zeros(L_HI, np.int64)
        hi_pos = hi_tile_base[w[hi]] * 128 + j[hi]
        idx_hi[hi_pos] = s[hi] - HI_BASE
        idx_r = np.zeros(E_PAD, np.int64)
        idx_r[pos] = r_loc
        idx_rank = np.zeros(E_PAD, np.int64)
        idx_rank[pos] = r_loc - w * 128

        nf_shard = np.zeros((NPC_PAD, 128), np.float32)
        nf_shard[:NPC] = node_features[c * NPC:(c + 1) * NPC]
        nfT_loc = np.zeros((128, NPC_PAD), np.float32)
        nfT_loc[:, :NPC] = node_features[c * NPC:(c + 1) * NPC].T

        m = dict(shared)
        m.update({
            "efT": efT,
            "rankT": rankT,
            "idx_lo": wrap_idx(idx_lo),
            "idx_hi": wrap_idx(idx_hi),
            "idx_r": wrap_idx(idx_r),
            "idx_rank": wrap_idx(idx_rank),
            "nf_shard": nf_shard,
            "nfT_loc": np.ascontiguousarray(nfT_loc),
        })
        in_maps.append(m)

    return structure, in_maps


# ----------------------------------------------------------------------------
# Bass kernel builder
# ----------------------------------------------------------------------------

def _emit_ln_store(nc, tc, wtiles, x, eps_sb, lnw_sb, lnb_sb, out_shard, w):
    """LayerNorm(x) * ln_w + ln_b -> out_shard[w*128:(w+1)*128]."""
    stats = wtiles.tile([128, 6], F32, tag="stats")
    nc.vector.bn_stats(out=stats[:], in_=x[:])
    mv = wtiles.tile([128, 2], F32, tag="mv")
    nc.vector.bn_aggr(out=mv[:], in_=stats[:])
    sd = wtiles.tile([128, 1], F32, tag="sd")
    nc.scalar.activation(
        out=sd[:], in_=mv[:, 1:2],
        func=mybir.ActivationFunctionType.Sqrt,
        bias=eps_sb[:], scale=1.0)
    rs = wtiles.tile([128, 1], F32, tag="rs")
    nc.vector.reciprocal(out=rs[:], in_=sd[:])
    xn = wtiles.tile([128, 128], F32, tag="xn")
    nc.vector.tensor_scalar(
        out=xn[:], in0=x[:], scalar1=mv[:, 0:1], scalar2=rs[:],
        op0=mybir.AluOpType.subtract, op1=mybir.AluOpType.mult)
    xw = wtiles.tile([128, 128], F32, tag="xw")
    nc.vector.tensor_mul(out=xw[:], in0=xn[:], in1=lnw_sb[:])
    ot = wtiles.tile([128, 128], F32, tag="ot")
    nc.vector.tensor_add(out=ot[:], in0=xw[:], in1=lnb_sb[:])
    nc.sync.dma_start(out=out_shard[w * 128:(w + 1) * 128, :], in_=ot[:])


def build_kernel(st, eps=1e-5, max_windows=None, use_gathers=True,
                 use_edge_mm=True, use_agg=True):
    N_PAD, NT_GLOBAL = st["N_PAD"], st["NT_GLOBAL"]
    NPC_PAD, WPC = st["NPC_PAD"], st["WPC"]
    NT, E_PAD = st["NT"], st["E_PAD"]
    T_lo, T_hi, T_w = st["T_lo"], st["T_hi"], st["T_w"]
    tile_base, lo_tile_base, hi_tile_base = (
        st["tile_base"], st["lo_tile_base"], st["hi_tile_base"])
    L_LO, L_HI, HI_BASE = st["L_LO"], st["L_HI"], st["HI_BASE"]
    T_MAX = int(T_w.max())

    nc = bacc.Bacc("TRN2", target_bir_lowering=False, debug=False)

    # inputs
    nfT = nc.dram_tensor("nfT", [128, N_PAD], F32, kind="ExternalInput")
    nfT_loc = nc.dram_tensor("nfT_loc", [128, NPC_PAD], F32, kind="ExternalInput")
    efT = nc.dram_tensor("efT", [128, E_PAD], F32, kind="ExternalInput")
    rankT = nc.dram_tensor("rankT", [128, NT], F32, kind="ExternalInput")
    idx_lo = nc.dram_tensor("idx_lo", [128, L_LO // 16], I16, kind="ExternalInput")
    idx_hi = nc.dram_tensor("idx_hi", [128, L_HI // 16], I16, kind="ExternalInput")
    idx_r = nc.dram_tensor("idx_r", [128, E_PAD // 16], I16, kind="ExternalInput")
    idx_rank = nc.dram_tensor("idx_rank", [128, E_PAD // 16], I16,
                              kind="ExternalInput")
    nf_shard = nc.dram_tensor("nf_shard", [NPC_PAD, 128], F32, kind="ExternalInput")
    W1 = nc.dram_tensor("W1", [128, 128], F32, kind="ExternalInput")
    W2 = nc.dram_tensor("W2", [128, 128], F32, kind="ExternalInput")
    W3 = nc.dram_tensor("W3", [128, 128], F32, kind="ExternalInput")
    b_bc = nc.dram_tensor("b_bc", [128, 128], F32, kind="ExternalInput")
    b_col = nc.dram_tensor("b_col", [128, 1], F32, kind="ExternalInput")
    ident = nc.dram_tensor("ident", [128, 128], F32, kind="ExternalInput")
    lnw_bc = nc.dram_tensor("lnw_bc", [128, 128], F32, kind="ExternalInput")
    lnb_bc = nc.dram_tensor("lnb_bc", [128, 128], F32, kind="ExternalInput")
    iota_in = nc.dram_tensor("iota", [128, 128], F32, kind="ExternalInput")

    # internal scratch + output
    P1 = nc.dram_tensor("P1", [N_PAD, 128], F32, kind="Internal")
    out_shard = nc.dram_tensor("out_shard", [NPC_PAD, 128], F32,
                               kind="ExternalOutput")

    with tile.TileContext(nc) as tc:
        with (
            tc.tile_pool(name="consts", bufs=1) as consts,
            tc.tile_pool(name="ptiles", bufs=4) as ptiles,
            tc.tile_pool(name="ppsum", bufs=4, space="PSUM") as ppsum,
            tc.tile_pool(name="ppsumb", bufs=2, space="PSUM") as ppsumb,
            tc.tile_pool(name="gtiles", bufs=3) as gtiles,
            tc.tile_pool(name="etile", bufs=3) as etile,
            tc.tile_pool(name="msgs", bufs=5) as msgs,
            tc.tile_pool(name="aggp", bufs=2, space="PSUM") as aggp,
            tc.tile_pool(name="wtiles", bufs=3) as wtiles,
        ):
            # constants in SBUF
            W1_sb = consts.tile([128, 128], F32)
            W2_sb = consts.tile([128, 128], F32)
            W3_sb = consts.tile([128, 128], F32)
            b_sb = consts.tile([128, 128], F32)
            lnw_sb = consts.tile([128, 128], F32)
            lnb_sb = consts.tile([128, 128], F32)
            iota_sb = consts.tile([128, 128], F32)
            bcol_sb = consts.tile([128, 1], F32)
            ident_sb = consts.tile([128, 128], F32)
            p2bT_sb = consts.tile([128, NPC_PAD], F32)
            eps_sb = consts.tile([128, 1], F32)
            for dst, src in ((W1_sb, W1), (W2_sb, W2), (W3_sb, W3),
                             (b_sb, b_bc), (lnw_sb, lnw_bc), (lnb_sb, lnb_bc),
                             (iota_sb, iota_in), (bcol_sb, b_col),
                             (ident_sb, ident)):
                nc.sync.dma_start(out=dst[:], in_=src[:])
            nc.vector.memset(eps_sb[:], eps)

            idxlo_sb = consts.tile([128, L_LO // 16], I16)
            idxhi_sb = consts.tile([128, L_HI // 16], I16)
            idxrank_sb = consts.tile([128, E_PAD // 16], I16)
            rankT_sb = consts.tile([128, NT], F32)
            nc.sync.dma_start(out=idxlo_sb[:], in_=idx_lo[:])
            nc.sync.dma_start(out=idxhi_sb[:], in_=idx_hi[:])
            nc.sync.dma_start(out=idxrank_sb[:], in_=idx_rank[:])
            nc.sync.dma_start(out=rankT_sb[:], in_=rankT[:])

            # ---------------- phase A: P1 = nf @ W1 (full, to DRAM) ----------
            for i0 in range(0, NT_GLOBAL, 4):
                k = min(4, NT_GLOBAL - i0)
                nf_t = ptiles.tile([128, 4 * 128], F32, tag="nf_t")
                nc.sync.dma_start(
                    out=nf_t[:, :k * 128],
                    in_=nfT[:, i0 * 128:(i0 + k) * 128])
                ps = ppsum.tile([128, 4, 128], F32, tag="pp")
                for t in range(k):
                    nc.tensor.matmul(
                        out=ps[:, t, :],
                        lhsT=nf_t[:, t * 128:(t + 1) * 128],
                        rhs=W1_sb[:],
                        start=True, stop=True)
                res = ptiles.tile([128, 4, 128], F32, tag="res")
                nc.scalar.copy(out=res[:, :k, :], in_=ps[:, :k, :])
                dst_ap = P1[i0 * 128:(i0 + k) * 128, :].rearrange(
                    "(t p) h -> p t h", p=128)
                nc.sync.dma_start(out=dst_ap, in_=res[:, :k, :])

            # ------ phase B: P2bT = (nf_loc @ W2 + b).T, kept in SBUF --------
            for j0 in range(0, NPC_PAD, 512):
                k = min(512, NPC_PAD - j0)
                nfl_t = ptiles.tile([128, 512], F32, tag="nfl_t")
                nc.sync.dma_start(out=nfl_t[:, :k], in_=nfT_loc[:, j0:j0 + k])
                psb = ppsumb.tile([128, 512], F32, tag="ppb")
                nc.tensor.matmul(
                    out=psb[:, :k], lhsT=W2_sb[:], rhs=nfl_t[:, :k],
                    start=True, stop=True)
                nc.scalar.add(out=p2bT_sb[:, j0:j0 + k], in_=psb[:, :k],
                              add=bcol_sb[:])

            # ---------------- phase C: edge loop -----------------------------
            n_win = WPC if max_windows is None else min(max_windows, WPC)
            for w in range(n_win):
                tw, tlo, thi = int(T_w[w]), int(T_lo[w]), int(T_hi[w])
                tb = int(tile_base[w])
                if tw == 0:
                    nf_w = wtiles.tile([128, 128], F32, tag="nfw")
                    nc.sync.dma_start(
                        out=nf_w[:], in_=nf_shard[w * 128:(w + 1) * 128, :])
                    x = wtiles.tile([128, 128], F32, tag="x")
                    nc.vector.tensor_copy(out=x[:], in_=nf_w[:])
                    _emit_ln_store(nc, tc, wtiles, x, eps_sb, lnw_sb, lnb_sb,
                                   out_shard, w)
                    continue
                g1 = gtiles.tile([128, T_MAX, 128], F32, tag="g1")
                if not use_gathers:
                    nc.vector.memset(g1[:, :tw, :], 0.0)
                if use_gathers and tlo > 0:
                    lb = int(lo_tile_base[w]) * 8   # 128/16 cols per tile
                    nc.gpsimd.dma_gather(
                        out_ap=g1[:, 0:tlo, :],
                        in_ap=P1[:, :],
                        idxs_ap=idxlo_sb[:, lb:lb + tlo * 8],
                        num_idxs=tlo * 128,
                        num_idxs_reg=tlo * 128,
                        elem_size=128, single_packet=False)
                if use_gathers and thi > 0:
                    hb = int(hi_tile_base[w]) * 8
                    nc.gpsimd.dma_gather(
                        out_ap=g1[:, tlo:tw, :],
                        in_ap=P1[HI_BASE:, :],
                        idxs_ap=idxhi_sb[:, hb:hb + thi * 8],
                        num_idxs=thi * 128,
                        num_idxs_reg=thi * 128,
                        elem_size=128, single_packet=False)
                p2x = gtiles.tile([128, T_MAX * 128], F32, tag="p2x")
                nc.gpsimd.ap_gather(
                    out_ap=p2x[:, :tw * 128].rearrange("p (n d) -> p n d", d=1),
                    in_ap=p2bT_sb[:, w * 128:(w + 1) * 128].rearrange(
                        "p (n d) -> p n d", d=1),
                    idxs_ap=idxrank_sb[:, tb * 8:(tb + tw) * 8],
                    channels=128, num_elems=128, d=1,
                    num_idxs=tw * 128)
                ef_sb = etile.tile([128, T_MAX * 128], F32, tag="ef")
                nc.sync.dma_start(
                    out=ef_sb[:, :tw * 128],
                    in_=efT[:, tb * 128:(tb + tw) * 128])

                agg = aggp.tile([128, 128], F32, tag="agg")
                if not use_agg:
                    nc.vector.memset(agg[:], 0.0)
                t_done = 0
                for c0 in range(0, tw, 4):
                    k = min(4, tw - c0)
                    pre = msgs.tile([128, 4, 128], F32, tag="pre")
                    for t in range(k):
                        ps = ppsum.tile([128, 128], F32, tag="pp")
                        if use_edge_mm:
                            # P2[r] lands first via PE transpose of the ap_gather
                            nc.tensor.matmul(
                                out=ps[:],
                                lhsT=p2x[:, (c0 + t) * 128:(c0 + t + 1) * 128],
                                rhs=ident_sb[:],
                                is_transpose=True,
                                start=True, stop=False,
                                skip_group_check=True)
                            nc.tensor.matmul(
                                out=ps[:],
                                lhsT=ef_sb[:, (c0 + t) * 128:(c0 + t + 1) * 128],
                                rhs=W3_sb[:],
                                start=False, stop=True,
                                skip_group_check=True)
                        else:
                            nc.vector.memset(ps[:], 0.0)
                        # pre = g1 + (EW + P2x)
                        nc.vector.tensor_add(
                            out=pre[:, t, :], in0=g1[:, c0 + t, :], in1=ps[:])
                    # msg = relu(pre)
                    msg = msgs.tile([128, 4, 128], F32, tag="msg")
                    nc.vector.tensor_scalar_max(
                        out=msg[:, :k, :], in0=pre[:, :k, :], scalar1=0.0)
                    # S one-hot + aggregation matmuls
                    S = msgs.tile([128, 4, 128], F32, tag="S")
                    iota_ap = iota_sb[:]
                    iota_bc = bass.AP(
                        tensor=iota_ap.tensor, offset=iota_ap.offset,
                        ap=[iota_ap.ap[0], [0, k], iota_ap.ap[1]])
                    rank_sl = rankT_sb[:, tb + c0:tb + c0 + k]
                    rank_bc = bass.AP(
                        tensor=rank_sl.tensor, offset=rank_sl.offset,
                        ap=[rank_sl.ap[0], rank_sl.ap[1], [0, 128]])
                    nc.vector.tensor_tensor(
                        out=S[:, :k, :], in0=iota_bc, in1=rank_bc,
                        op=mybir.AluOpType.is_equal)
                    if use_agg:
                        for t in range(k):
                            nc.tensor.matmul(
                                out=agg[:],
                                lhsT=S[:, t, :],
                                rhs=msg[:, t, :],
                                start=(t_done == 0), stop=(t_done == tw - 1),
                                skip_group_check=True)
                            t_done += 1

                # residual + LayerNorm
                nf_w = wtiles.tile([128, 128], F32, tag="nfw")
                nc.sync.dma_start(out=nf_w[:], in_=nf_shard[w * 128:(w + 1) * 128, :])
                x = wtiles.tile([128, 128], F32, tag="x")
                nc.vector.tensor_add(out=x[:], in0=agg[:], in1=nf_w[:])
                _emit_ln_store(nc, tc, wtiles, x, eps_sb, lnw_sb, lnb_sb,
                               out_shard, w)

    nc.compile()
    return nc


# ----------------------------------------------------------------------------
# Full entry: host prep + device run + assembly
# ----------------------------------------------------------------------------

def run(node_features, senders, receivers, edge_features, W, b, ln_w, ln_b,
        n_cores=8, return_nc=False):
    from concourse.bass_utils import run_bass_kernel_spmd
    st, in_maps = host_prep(node_features, senders, receivers, edge_features,
                            W, b, ln_w, ln_b, n_cores)
    nc = build_kernel(st)
    res = run_bass_kernel_spmd(nc, in_maps, core_ids=list(range(n_cores)))
    NPC = st["NPC"]
    out = np.concatenate(
        [res.results[c]["out_shard"][:NPC] for c in range(n_cores)], axis=0)
    if return_nc:
        return out, nc, st, in_maps
    return out


# ----------------------------------------------------------------------------
# Harness entry point
# ----------------------------------------------------------------------------

def kernel(**inputs):
    """Full-input entry: shards across 8 NeuronCores internally."""
    out = run(
        node_features=np.asarray(inputs["node_features"], np.float32),
        senders=np.asarray(inputs["senders"], np.int32),
        receivers=np.asarray(inputs["receivers"], np.int32),
        edge_features=np.asarray(inputs["edge_features"], np.float32),
        W=np.asarray(inputs["W"], np.float32),
        b=np.asarray(inputs["b"], np.float32),
        ln_w=np.asarray(inputs["ln_w"], np.float32),
        ln_b=np.asarray(inputs["ln_b"], np.float32),
        n_cores=8,
    )
    return out.astype(np.float32)



# revision 5
# speedup vs baseline: 2.2628x; 2.2628x over previous
"""GNN message-passing kernel for Trainium2 (Bass/Tile), 8-core SPMD.

Sharding: edges sharded by receiver range (edge/data parallel, no collectives).
Core c owns receivers in [c*NPC, (c+1)*NPC). Host stages a per-edge bf16
stream esT = [efT | nf[senders]T] interleaved per 128-edge tile, so the
sender gather happens at staging time and the device only streams it.

Per core:
  phase B: p2bT = (nf_loc @ W2 + b)^T kept in SBUF (fp32)
  phase C: per 128-node window w, per 128-edge tile t:
    ps  = P2b[r]^T^T                      (windowed fp32 ap_gather + fp32
                                           is_transpose into PSUM)
    ps += nfs_t @ W1 + ef_t @ W3          (two bf16 matmuls)
    msg = relu(ps) -> bf16                (ACT, batched over 4 tiles)
    S   = (iota == rank_col) -> bf16      (DVE tensor_scalar is_equal)
    agg += S^T @ msg                      (bf16 matmul, PSUM accum per window)
  out = LayerNorm(agg + nf_shard)         (fused stats on DVE/ACT)
"""

import numpy as np

import concourse.bacc as bacc
import concourse.tile as tile
import concourse.mybir as mybir
import concourse.bass as bass

F32 = mybir.dt.float32
BF16 = mybir.dt.bfloat16
I16 = mybir.dt.int16

AF = mybir.ActivationFunctionType
ALU = mybir.AluOpType


def _to_bf16(x):
    import ml_dtypes
    return x.astype(ml_dtypes.bfloat16)


# ----------------------------------------------------------------------------
# Host-side preparation
# ----------------------------------------------------------------------------

def host_prep(node_features, senders, receivers, edge_features, W, b, ln_w, ln_b,
              n_cores=8):
    N, H = node_features.shape
    E = senders.shape[0]
    assert H == 128
    NPC = N // n_cores                      # nodes per core
    WPC = (NPC + 127) // 128                # windows per core
    NPC_PAD = WPC * 128

    node_features = np.asarray(node_features, np.float32)
    senders = np.asarray(senders, np.int32)
    receivers = np.asarray(receivers, np.int32)
    edge_features = np.asarray(edge_features, np.float32)

    core_of_edge = np.minimum(receivers // NPC, n_cores - 1)

    # ---- pass 1: per-core, per-window counts --------------------------------
    per_core = []
    cnt = np.zeros((n_cores, WPC), np.int64)
    for c in range(n_cores):
        sel = np.nonzero(core_of_edge == c)[0]
        r_loc = receivers[sel] - c * NPC
        w = r_loc // 128
        order = np.argsort(w, kind="stable")
        sel, r_loc, w = sel[order], r_loc[order], w[order]
        cnt[c] = np.bincount(w, minlength=WPC)
        per_core.append((sel, r_loc, w))

    T_w = ((cnt.max(axis=0) + 127) // 128).astype(np.int64)   # tiles per window
    NT = int(T_w.sum())
    E_PAD = NT * 128
    tile_base = np.concatenate(([0], np.cumsum(T_w)[:-1]))

    ln_trivial = bool(np.allclose(ln_w, 1.0) and np.allclose(ln_b, 0.0))

    structure = dict(N=N, H=H, E=E, NPC=NPC, WPC=WPC, NPC_PAD=NPC_PAD,
                     NT=NT, E_PAD=E_PAD, T_w=T_w, tile_base=tile_base,
                     ln_trivial=ln_trivial)

    def wrap_idx(arr):
        """int16 stream -> [128, L/16] wrapped layout (replicated per 16 rows)."""
        L = arr.shape[0]
        assert L % 16 == 0
        w16 = arr.reshape(-1, 16).T.astype(np.int16)   # [16, L/16]
        return np.ascontiguousarray(np.tile(w16, (8, 1)))

    # ---- shared (core-independent) inputs -----------------------------------
    W1 = np.ascontiguousarray(W[0:128], np.float32)
    W2 = np.ascontiguousarray(W[128:256], np.float32)
    W3 = np.ascontiguousarray(W[256:384], np.float32)
    iota_row = np.broadcast_to(np.arange(128, dtype=np.float32), (128, 128))
    shared = {
        "W1b": _to_bf16(W1),
        "W2b": _to_bf16(W2),
        "W3b": _to_bf16(W3),
        "b_col": np.asarray(b, np.float32).reshape(128, 1).copy(),
        "iota16": _to_bf16(iota_row.copy()),
        "ident": np.eye(128, dtype=np.float32),
        "lnw_bc": np.broadcast_to(np.asarray(ln_w, np.float32), (128, 128)).copy(),
        "lnb_bc": np.broadcast_to(np.asarray(ln_b, np.float32), (128, 128)).copy(),
    }

    # ---- pass 2: per-core streams -------------------------------------------
    in_maps = []
    for c in range(n_cores):
        sel, r_loc, w = per_core[c]
        Ec = sel.shape[0]
        # position of each edge in the padded stream
        starts = np.concatenate(([0], np.nonzero(np.diff(w))[0] + 1))
        grp_start = np.repeat(starts, np.diff(np.concatenate((starts, [Ec]))))
        j = np.arange(Ec) - grp_start
        pos = tile_base[w] * 128 + j

        ef_pad = np.zeros((E_PAD, 128), np.float32)
        ef_pad[pos] = edge_features[sel]
        nfs_pad = np.zeros((E_PAD, 128), np.float32)
        nfs_pad[pos] = node_features[senders[sel]]

        # esT [128, 2*E_PAD] bf16: per tile t, cols [2t*128,(2t+1)*128) = efT,
        # cols [(2t+1)*128,(2t+2)*128) = nf[s]T
        esT = np.empty((128, NT, 2, 128), np.float32)
        esT[:, :, 0, :] = ef_pad.reshape(NT, 128, 128).transpose(2, 0, 1)
        esT[:, :, 1, :] = nfs_pad.reshape(NT, 128, 128).transpose(2, 0, 1)
        esT = _to_bf16(np.ascontiguousarray(esT.reshape(128, 2 * E_PAD)))

        rank = np.full(E_PAD, -1.0, np.float32)
        rank[pos] = (r_loc - w * 128).astype(np.float32)
        rankT = np.ascontiguousarray(rank.reshape(NT, 128).T)   # [128, NT]

        idx_rank = np.zeros(E_PAD, np.int64)
        idx_rank[pos] = r_loc - w * 128

        nf_shard = np.zeros((NPC_PAD, 128), np.float32)
        nf_shard[:NPC] = node_features[c * NPC:(c + 1) * NPC]
        nfT_loc = np.zeros((128, NPC_PAD), np.float32)
        nfT_loc[:, :NPC] = node_features[c * NPC:(c + 1) * NPC].T

        m = dict(shared)
        m.update({
            "esT": esT,
            "rankT": rankT,
            "idx_rank": wrap_idx(idx_rank),
            "nf_shard": nf_shard,
            "nfT_loc": _to_bf16(np.ascontiguousarray(nfT_loc)),
        })
        in_maps.append(m)

    return structure, in_maps


# ----------------------------------------------------------------------------
# Bass kernel builder
# ----------------------------------------------------------------------------

def build_kernel(st, eps=1e-5, max_windows=None):
    NPC_PAD, WPC = st["NPC_PAD"], st["WPC"]
    NT, E_PAD = st["NT"], st["E_PAD"]
    T_w, tile_base = st["T_w"], st["tile_base"]
    ln_trivial = st["ln_trivial"]
    T_MAX = int(T_w.max())
    # es load batches of 2 windows
    T2 = [int(T_w[i] + (T_w[i + 1] if i + 1 < WPC else 0))
          for i in range(0, WPC, 2)]
    T2_MAX = max(T2)

    nc = bacc.Bacc("TRN2", target_bir_lowering=False, debug=False)

    # inputs
    esT = nc.dram_tensor("esT", [128, 2 * E_PAD], BF16, kind="ExternalInput")
    rankT = nc.dram_tensor("rankT", [128, NT], F32, kind="ExternalInput")
    idx_rank = nc.dram_tensor("idx_rank", [128, E_PAD // 16], I16,
                              kind="ExternalInput")
    nfT_loc = nc.dram_tensor("nfT_loc", [128, NPC_PAD], BF16, kind="ExternalInput")
    nf_shard = nc.dram_tensor("nf_shard", [NPC_PAD, 128], F32, kind="ExternalInput")
    W1b = nc.dram_tensor("W1b", [128, 128], BF16, kind="ExternalInput")
    W2b = nc.dram_tensor("W2b", [128, 128], BF16, kind="ExternalInput")
    W3b = nc.dram_tensor("W3b", [128, 128], BF16, kind="ExternalInput")
    b_col = nc.dram_tensor("b_col", [128, 1], F32, kind="ExternalInput")
    iota16 = nc.dram_tensor("iota16", [128, 128], BF16, kind="ExternalInput")
    ident = nc.dram_tensor("ident", [128, 128], F32, kind="ExternalInput")
    lnw_bc = nc.dram_tensor("lnw_bc", [128, 128], F32, kind="ExternalInput")
    lnb_bc = nc.dram_tensor("lnb_bc", [128, 128], F32, kind="ExternalInput")

    out_shard = nc.dram_tensor("out_shard", [NPC_PAD, 128], F32,
                               kind="ExternalOutput")

    with tile.TileContext(nc) as tc:
        with (
            tc.tile_pool(name="consts", bufs=1) as consts,
        ):
            # constants in SBUF
            W1_sb = consts.tile([128, 128], BF16)
            W2_sb = consts.tile([128, 128], BF16)
            W3_sb = consts.tile([128, 128], BF16)
            iota_sb = consts.tile([128, 128], BF16)
            ident_sb = consts.tile([128, 128], F32)
            bcol_sb = consts.tile([128, 1], F32)
            lnw_sb = consts.tile([128, 128], F32)
            lnb_sb = consts.tile([128, 128], F32)
            eps_sb = consts.tile([128, 1], F32)
            rankT_sb = consts.tile([128, NT], F32)
            idxr_sb = consts.tile([128, E_PAD // 16], I16)
            p2bT = consts.tile([128, NPC_PAD], F32)

            loads = [(W1_sb, W1b), (W2_sb, W2b), (W3_sb, W3b),
                     (iota_sb, iota16), (ident_sb, ident), (bcol_sb, b_col),
                     (rankT_sb, rankT), (idxr_sb, idx_rank)]
            if not ln_trivial:
                loads += [(lnw_sb, lnw_bc), (lnb_sb, lnb_bc)]
            for dst, src in loads:
                nc.sync.dma_start(out=dst[:], in_=src[:])
            nc.vector.memset(eps_sb[:], eps)

            # ---- phase B: p2bT = (nf_loc @ W2 + b)^T in SBUF ----------------
            with (
                tc.tile_pool(name="pb", bufs=2) as pb,
                tc.tile_pool(name="pbps", bufs=2, space="PSUM") as pbps,
            ):
                for j0 in range(0, NPC_PAD, 512):
                    k = min(512, NPC_PAD - j0)
                    nfl = pb.tile([128, 512], BF16, tag="nfl")
                    nc.sync.dma_start(out=nfl[:, :k], in_=nfT_loc[:, j0:j0 + k])
                    psA = pbps.tile([128, 512], F32, tag="psA")
                    nc.tensor.matmul(out=psA[:, :k], lhsT=W2_sb[:],
                                     rhs=nfl[:, :k], start=True, stop=True)
                    nc.scalar.activation(
                        out=p2bT[:, j0:j0 + k], in_=psA[:, :k],
                        func=AF.Identity, bias=bcol_sb[:], scale=1.0)

            # ---- phase C: edge loop -----------------------------------------
            with (
                tc.tile_pool(name="es", bufs=2) as espool,
                tc.tile_pool(name="p2xp", bufs=2) as p2xp,
                tc.tile_pool(name="msgs", bufs=3) as msgs,
                tc.tile_pool(name="ppsum", bufs=3, space="PSUM") as ppsum,
                tc.tile_pool(name="aggp", bufs=2, space="PSUM") as aggp,
                tc.tile_pool(name="wt", bufs=4) as wt,
                tc.tile_pool(name="outp", bufs=2) as outp,
            ):
                n_win = WPC if max_windows is None else min(max_windows, WPC)
                es_sb = None
                ot_batch = None
                for w in range(n_win):
                    tw = int(T_w[w])
                    tb = int(tile_base[w])
                    if w % 2 == 0:
                        # load es for windows w, w+1
                        bt = T2[w // 2]
                        es_sb = espool.tile([128, 2 * T2_MAX * 128], BF16,
                                            tag="es")
                        nc.sync.dma_start(
                            out=es_sb[:, :2 * bt * 128],
                            in_=esT[:, 2 * tb * 128:2 * (tb + bt) * 128])
                        es_off = 0
                    else:
                        es_off = 2 * int(T_w[w - 1]) * 128

                    # output batch tile (4 windows per store)
                    if w % 4 == 0:
                        ot_batch = outp.tile([128, 4, 128], F32, tag="ot")

                    if tw > 0:
                        p2x = p2xp.tile([128, T_MAX * 128], F32, tag="p2x")
                        nc.gpsimd.ap_gather(
                            out_ap=p2x[:, :tw * 128].rearrange(
                                "p (n d) -> p n d", d=1),
                            in_ap=p2bT[:, w * 128:(w + 1) * 128].rearrange(
                                "p (n d) -> p n d", d=1),
                            idxs_ap=idxr_sb[:, tb * 8:(tb + tw) * 8],
                            channels=128, num_elems=128, d=1,
                            num_idxs=tw * 128)

                        agg = aggp.tile([128, 128], F32, tag="agg")
                        t_done = 0
                        for c0 in range(0, tw, 4):
                            k = min(4, tw - c0)
                            ps = ppsum.tile([128, 4, 128], F32, tag="ps")
                            for t in range(k):
                                g = c0 + t
                                nc.tensor.matmul(
                                    out=ps[:, t, :],
                                    lhsT=p2x[:, g * 128:(g + 1) * 128],
                                    rhs=ident_sb[:],
                                    is_transpose=True,
                                    start=True, stop=False,
                                    skip_group_check=True)
                                nc.tensor.matmul(
                                    out=ps[:, t, :],
                                    lhsT=es_sb[:, es_off + (2 * g + 1) * 128:
                                               es_off + (2 * g + 2) * 128],
                                    rhs=W1_sb[:],
                                    start=False, stop=False,
                                    skip_group_check=True)
                                nc.tensor.matmul(
                                    out=ps[:, t, :],
                                    lhsT=es_sb[:, es_off + 2 * g * 128:
                                               es_off + (2 * g + 1) * 128],
                                    rhs=W3_sb[:],
                                    start=False, stop=True,
                                    skip_group_check=True)
                            msg = msgs.tile([128, 4, 128], BF16, tag="msg")
                            nc.scalar.activation(
                                out=msg[:, :k, :], in_=ps[:, :k, :],
                                func=AF.Relu, scale=1.0)
                            S = msgs.tile([128, 4, 128], BF16, tag="S")
                            for t in range(k):
                                nc.vector.tensor_scalar(
                                    out=S[:, t, :], in0=iota_sb[:],
                                    scalar1=rankT_sb[:, tb + c0 + t:
                                                     tb + c0 + t + 1],
                                    scalar2=None, op0=ALU.is_equal)
                            for t in range(k):
                                nc.tensor.matmul(
                                    out=agg[:],
                                    lhsT=S[:, t, :],
                                    rhs=msg[:, t, :],
                                    start=(t_done == 0), stop=(t_done == tw - 1),
                                    skip_group_check=True)
                                t_done += 1

                    # ---- residual + LayerNorm -------------------------------
                    nf_w = wt.tile([128, 128], F32, tag="nfw")
                    nc.sync.dma_start(out=nf_w[:],
                                      in_=nf_shard[w * 128:(w + 1) * 128, :])
                    x = wt.tile([128, 128], F32, tag="x")
                    s1 = wt.tile([128, 1], F32, tag="s1")
                    if tw > 0:
                        nc.vector.scalar_tensor_tensor(
                            out=x[:], in0=agg[:], scalar=0.0, in1=nf_w[:],
                            op0=ALU.add, op1=ALU.add, accum_out=s1[:])
                    else:
                        nc.vector.scalar_tensor_tensor(
                            out=x[:], in0=nf_w[:], scalar=0.0, in1=nf_w[:],
                            op0=ALU.mult, op1=ALU.add, accum_out=s1[:])
                    xsq = wt.tile([128, 128], F32, tag="xsq")
                    s2 = wt.tile([128, 1], F32, tag="s2")
                    nc.vector.scalar_tensor_tensor(
                        out=xsq[:], in0=x[:], scalar=1.0, in1=x[:],
                        op0=ALU.mult, op1=ALU.mult, accum_out=s2[:])
                    nmu = wt.tile([128, 1], F32, tag="nmu")
                    nc.vector.tensor_scalar(
                        out=nmu[:], in0=s1[:], scalar1=-1.0 / 128.0,
                        scalar2=None, op0=ALU.mult)
                    mu2 = wt.tile([128, 1], F32, tag="mu2")
                    nc.vector.tensor_mul(out=mu2[:], in0=nmu[:], in1=nmu[:])
                    var = wt.tile([128, 1], F32, tag="var")
                    nc.vector.tensor_scalar(
                        out=var[:], in0=s2[:], scalar1=1.0 / 128.0,
                        scalar2=mu2[:], op0=ALU.mult, op1=ALU.subtract)
                    sd = wt.tile([128, 1], F32, tag="sd")
                    nc.scalar.activation(out=sd[:], in_=var[:], func=AF.Sqrt,
                                         bias=eps_sb[:], scale=1.0)
                    rs = wt.tile([128, 1], F32, tag="rs")
                    nc.vector.reciprocal(out=rs[:], in_=sd[:])
                    nmurs = wt.tile([128, 1], F32, tag="nmurs")
                    nc.vector.tensor_mul(out=nmurs[:], in0=nmu[:], in1=rs[:])
                    if ln_trivial:
                        nc.scalar.activation(
                            out=ot_batch[:, w % 4, :], in_=x[:],
                            func=AF.Identity, bias=nmurs[:], scale=rs[:])
                    else:
                        xn = wt.tile([128, 128], F32, tag="xn")
                        nc.scalar.activation(
                            out=xn[:], in_=x[:],
                            func=AF.Identity, bias=nmurs[:], scale=rs[:])
                        xw = wt.tile([128, 128], F32, tag="xw")
                        nc.vector.tensor_mul(out=xw[:], in0=xn[:], in1=lnw_sb[:])
                        nc.vector.tensor_add(out=ot_batch[:, w % 4, :],
                                             in0=xw[:], in1=lnb_sb[:])
                    if w % 4 == 3 or w == n_win - 1:
                        w0 = (w // 4) * 4
                        kw = w - w0 + 1
                        dst = out_shard[w0 * 128:(w0 + kw) * 128, :].rearrange(
                            "(t p) h -> p t h", p=128)
                        nc.sync.dma_start(out=dst, in_=ot_batch[:, :kw, :])

    nc.compile()
    return nc


# ----------------------------------------------------------------------------
# Full entry: host prep + device run + assembly
# ----------------------------------------------------------------------------

def run(node_features, senders, receivers, edge_features, W, b, ln_w, ln_b,
        n_cores=8, return_nc=False):
    from concourse.bass_utils import run_bass_kernel_spmd
    st, in_maps = host_prep(node_features, senders, receivers, edge_features,
                            W, b, ln_w, ln_b, n_cores)
    nc = build_kernel(st)
    res = run_bass_kernel_spmd(nc, in_maps, core_ids=list(range(n_cores)))
    NPC = st["NPC"]
    out = np.concatenate(
        [res.results[c]["out_shard"][:NPC] for c in range(n_cores)], axis=0)
    if return_nc:
        return out, nc, st, in_maps
    return out


# ----------------------------------------------------------------------------
# Harness entry point
# ----------------------------------------------------------------------------

def kernel(**inputs):
    """Full-input entry: shards across 8 NeuronCores internally."""
    out = run(
        node_features=np.asarray(inputs["node_features"], np.float32),
        senders=np.asarray(inputs["senders"], np.int32),
        receivers=np.asarray(inputs["receivers"], np.int32),
        edge_features=np.asarray(inputs["edge_features"], np.float32),
        W=np.asarray(inputs["W"], np.float32),
        b=np.asarray(inputs["b"], np.float32),
        ln_w=np.asarray(inputs["ln_w"], np.float32),
        ln_b=np.asarray(inputs["ln_b"], np.float32),
        n_cores=8,
    )
    return out.astype(np.float32)
